# revision 6
# baseline (speedup 1.0000x reference)
"""ActTransNet Trainium2 kernel: 8-core SPMD, routing done host-side.

Network (B=1024, T=16, INPUT_DIM=2048, DIM=1024, N_ACTIONS=64):
    p_avg = mean_t(precondition);  e_avg = mean_t(effect)
    p_embed = p_avg @ Wp.T + bp;   e_embed = e_avg @ We.T + be
    p_t[b]  = W_trans[action[b]] @ p_embed[b]
    returns (p_t[:, None, :, None], e_embed)

Sharding strategy (hardcoded): sort samples by action on the host, split the
sorted batch into 8 contiguous chunks of 128 samples.  Each core receives its
chunk of precondition/effect, the K distinct expert matrices its chunk spans
(K = max over cores, zero-masked where unused), a one-hot sample->slot mask,
and full (replicated) projection weights.  The expert matvec is computed as
sum_k (p_embed * mask_k) @ W_sel[k].T accumulated in PSUM, which keeps the
program static SPMD while each core reads only ~K/64 of W_trans.

Device pipeline per core:
  1. mean-pool over T rides the DMA (CCE accumulate chains); host supplies x
     as [T, J, S] so the pooled result lands directly in [j, s] matmul layout
  2. projections contract over j with the pooled tile as stationary operand
     and W.T streamed as the moving operand; the bias is folded into the same
     PSUM accumulation group as a rank-1 ones^T @ b matmul
  3. p_embed is PE-transposed to [d, s]; masked per expert slot; expert
     matmuls accumulate sum_k over K slots and 8 d-tiles into one PSUM pair
"""

import sys

try:  # concourse is on sys.path in the axon images; fall back to the repo
    import concourse.bass  # noqa: F401
except ImportError:  # pragma: no cover
    sys.path.insert(0, "/opt/trn_rl_repo")

import numpy as np

import concourse.bass as bass
import concourse.mybir as mybir
import concourse.tile as tile
from concourse import bacc, bass2jax
from concourse.masks import make_identity

# Problem shape (hardcoded per contest rules)
B, T, J, D, NA = 1024, 16, 2048, 1024, 64
NC = 8            # cores
S = B // NC       # samples per core = 128
P = 128           # partitions
JT = J // P       # 16 j-tiles (input dim)
DT = D // P       # 8 d-tiles (embed dim)
NH = D // 512     # 2 psum-bank halves of the embed dim
F32 = mybir.dt.float32

_kernel_cache: dict = {}


def _build(K: int, n_chains: int = 4):
    """Build the SPMD Bass program for K expert slots per core."""
    nc = bacc.Bacc(None, target_bir_lowering=False, debug=False, num_devices=NC)

    xp = nc.declare_dram_parameter("xp", [T, J, S], F32, isOutput=False)
    xe = nc.declare_dram_parameter("xe", [T, J, S], F32, isOutput=False)
    wp = nc.declare_dram_parameter("wp", [J, D], F32, isOutput=False)
    we = nc.declare_dram_parameter("we", [J, D], F32, isOutput=False)
    bp = nc.declare_dram_parameter("bp", [1, D], F32, isOutput=False)
    be = nc.declare_dram_parameter("be", [1, D], F32, isOutput=False)
    wsel = nc.declare_dram_parameter("wsel", [K, D, D], F32, isOutput=False)
    mask = nc.declare_dram_parameter("mask", [K, P, S], F32, isOutput=False)
    pt = nc.declare_dram_parameter("pt", [S, D], F32, isOutput=True)
    eo = nc.declare_dram_parameter("eo", [S, D], F32, isOutput=True)

    with tile.TileContext(nc) as tc:
        with (
            tc.tile_pool(name="acc", bufs=2 * n_chains) as accp,
            tc.tile_pool(name="pooled", bufs=2) as poolp,
            tc.tile_pool(name="wstream", bufs=6) as wsp,
            tc.tile_pool(name="emb", bufs=2) as embp,
            tc.tile_pool(name="embT", bufs=1) as embTp,
            tc.tile_pool(name="small", bufs=4) as smallp,
            tc.tile_pool(name="maskp", bufs=1) as maskp,
            tc.tile_pool(name="xm", bufs=3) as xmp,
            tc.tile_pool(name="out", bufs=2) as outp,
            tc.tile_pool(name="ps", bufs=2, space="PSUM") as psp,
            tc.tile_pool(name="tps", bufs=2, space="PSUM") as tpsp,
            tc.tile_pool(name="ps2", bufs=1, space="PSUM") as ps2p,
        ):
            ident = smallp.tile([P, P], F32, tag="ident")
            make_identity(nc, ident[:])
            ones = smallp.tile([1, P], F32, tag="ones")
            nc.gpsimd.memset(ones[:], 1.0)

            emb_tiles = {}
            for name, x_dram, w_dram, b_dram in (
                ("p", xp, wp, bp),
                ("e", xe, we, be),
            ):
                # ---- mean-pool over T, riding the DMA (CCE accumulate) ----
                # slab t: x[t] is [J, S]; view as [P, jt, s] so partition dim
                # is j-within-tile and the free layout matches paT below.
                chains = []
                per = T // n_chains
                for q in range(n_chains):
                    acc = accp.tile([P, JT * S], F32, tag="acc")
                    for i in range(per):
                        t = q * per + i
                        slab = x_dram[t].rearrange("(jt p) s -> p jt s", p=P)
                        nc.gpsimd.dma_start(
                            acc[:],
                            slab,
                            accum_op=(
                                mybir.AluOpType.bypass
                                if i == 0
                                else mybir.AluOpType.add
                            ),
                        )
                    chains.append(acc)
                # merge chains -> pooled-transposed [j, s] per j-tile
                while len(chains) > 1:
                    nxt = []
                    for a, b2 in zip(chains[0::2], chains[1::2]):
                        if len(chains) == 2:
                            m = poolp.tile([P, JT * S], F32, tag="pooledT")
                        else:
                            m = accp.tile([P, JT * S], F32, tag="acc")
                        nc.vector.tensor_add(m[:], a[:], b2[:])
                        nxt.append(m)
                    chains = nxt
                paT = chains[0]  # [P, (jt, s)]: sum over T (mean folded in W)

                bsb = smallp.tile([1, D], F32, tag="bias")
                nc.sync.dma_start(bsb[:], b_dram[:])

                # ---- projection: emb[s, d] = paT.T @ (W.T/16) + ones.T @ b
                emb = embp.tile([P, D], F32, tag="emb")
                psh = []
                for h in range(NH):
                    psh.append(psp.tile([P, 512], F32, tag="ps", name=f"ps{h}"))
                for jt in range(JT):
                    wst = wsp.tile([P, D], F32, tag="w")
                    nc.sync.dma_start(
                        wst[:], w_dram[jt * P : (jt + 1) * P, :]
                    )
                    for h in range(NH):
                        nc.tensor.matmul(
                            psh[h][:],
                            paT[:, jt * S : (jt + 1) * S],
                            wst[:, h * 512 : (h + 1) * 512],
                            start=(jt == 0),
                            stop=False,
                        )
                for h in range(NH):
                    # bias via rank-1 matmul: ones^T (1xS) @ b (1x512)
                    nc.tensor.matmul(
                        psh[h][:],
                        ones[:],
                        bsb[:, h * 512 : (h + 1) * 512],
                        start=False,
                        stop=True,
                    )
                    nc.scalar.activation(
                        emb[:, h * 512 : (h + 1) * 512],
                        psh[h][:],
                        mybir.ActivationFunctionType.Identity,
                    )
                emb_tiles[name] = emb

            # e_embed: straight out
            nc.sync.dma_start(eo[:], emb_tiles["e"][:])

            # ---- transpose p_embed [s, d] -> pembT [d, s] per d-tile ----
            pembT = embTp.tile([P, DT * S], F32, tag="pembT")
            for dt in range(DT):
                tp = tpsp.tile([P, P], F32, tag="tps")
                nc.tensor.transpose(
                    tp[:], emb_tiles["p"][:, dt * P : (dt + 1) * P], ident[:]
                )
                nc.scalar.activation(
                    pembT[:, dt * S : (dt + 1) * S],
                    tp[:],
                    mybir.ActivationFunctionType.Identity,
                )

            # ---- expert transform: pt[s, i] = sum_k (pembT*m_k)^T @ Wsel_k
            msb = maskp.tile([P, K * S], F32, tag="mask")
            nc.sync.dma_start(msb[:], mask.rearrange("k p s -> p k s"))

            pspt = ps2p.tile([P, D], F32, tag="pspt")
            for k in range(K):
                xm = xmp.tile([P, DT * S], F32, tag="xm")
                for dt in range(DT):
                    nc.vector.tensor_mul(
                        xm[:, dt * S : (dt + 1) * S],
                        pembT[:, dt * S : (dt + 1) * S],
                        msb[:, k * S : (k + 1) * S],
                    )
                for dt in range(DT):
                    wst = wsp.tile([P, D], F32, tag="w")
                    nc.sync.dma_start(
                        wst[:], wsel[k, dt * P : (dt + 1) * P, :]
                    )
                    first = k == 0 and dt == 0
                    last = k == K - 1 and dt == DT - 1
                    for h in range(NH):
                        nc.tensor.matmul(
                            pspt[:, h * 512 : (h + 1) * 512],
                            xm[:, dt * S : (dt + 1) * S],
                            wst[:, h * 512 : (h + 1) * 512],
                            start=first,
                            stop=last,
                        )
            ptsb = outp.tile([P, D], F32, tag="pt")
            nc.vector.tensor_copy(ptsb[:], pspt[:])
            nc.sync.dma_start(pt[:], ptsb[:])

    nc.compile()
    return nc


def _prep(precondition, effect, action, Wp, bp, We, be, W_trans):
    """Host-side routing + layout prep. Returns (in_maps, perm, K)."""
    act = np.asarray(action).astype(np.int64).ravel()
    perm = np.argsort(act, kind="stable")
    act_sorted = act[perm]

    # per-core distinct actions and sample->slot segmentation
    chunk_acts = act_sorted.reshape(NC, S)
    uniq = [np.unique(ca) for ca in chunk_acts]
    K = max(len(u) for u in uniq)

    xs_p = np.asarray(precondition, dtype=np.float32)[perm]
    xs_e = np.asarray(effect, dtype=np.float32)[perm]
    # [B, T, J] -> [T*J, B] (one cache-friendly 2D transpose), view [T, J, B]
    xt_p = np.ascontiguousarray(xs_p.reshape(B, T * J).T).reshape(T, J, B)
    xt_e = np.ascontiguousarray(xs_e.reshape(B, T * J).T).reshape(T, J, B)

    scale = np.float32(1.0 / T)
    wp_t = np.ascontiguousarray(np.asarray(Wp, np.float32).T) * scale
    we_t = np.ascontiguousarray(np.asarray(We, np.float32).T) * scale
    bp_ = np.asarray(bp, np.float32).reshape(1, D).copy()
    be_ = np.asarray(be, np.float32).reshape(1, D).copy()
    # rhs convention needs W^T ([j, i]); transpose once globally
    Wt = np.ascontiguousarray(np.asarray(W_trans, np.float32).transpose(0, 2, 1))

    in_maps = []
    for c in range(NC):
        ca = chunk_acts[c]
        u = uniq[c]
        sel = np.zeros((K, D, D), np.float32)
        sel[: len(u)] = Wt[u]
        slot = np.searchsorted(u, ca)  # [S] slot index per sample
        m = np.zeros((K, S), np.float32)
        m[slot, np.arange(S)] = 1.0
        # replicate mask across the 128 j-partitions: [K, P, S]
        mb = np.ascontiguousarray(
            np.broadcast_to(m[:, None, :], (K, P, S))
        )
        in_maps.append(
            {
                "xp": np.ascontiguousarray(xt_p[:, :, c * S : (c + 1) * S]),
                "xe": np.ascontiguousarray(xt_e[:, :, c * S : (c + 1) * S]),
                "wp": wp_t,
                "we": we_t,
                "bp": bp_,
                "be": be_,
                "wsel": sel,
                "mask": mb,
            }
        )
    return in_maps, perm, K


def kernel(precondition, effect, action, Wp, bp, We, be, W_trans):
    in_maps, perm, K = _prep(
        precondition, effect, action, Wp, bp, We, be, W_trans
    )
    nc = _kernel_cache.get(K)
    if nc is None:
        nc = _build(K)
        _kernel_cache[K] = nc

    results = bass2jax.run_bass_via_pjrt(nc, in_maps, n_cores=NC)

    p_sorted = np.concatenate([np.asarray(r["pt"]) for r in results], axis=0)
    e_sorted = np.concatenate([np.asarray(r["eo"]) for r in results], axis=0)
    inv = np.empty_like(perm)
    inv[perm] = np.arange(B)
    p_full = p_sorted[inv]
    e_full = e_sorted[inv]
    return (p_full[:, None, :, None].astype(np.float32),
            e_full.astype(np.float32))


# revision 7
# speedup vs baseline: 2.8697x; 2.8697x over previous
"""ActTransNet Trainium2 kernel: 8-core SPMD, routing done host-side.

Network (B=1024, T=16, INPUT_DIM=2048, DIM=1024, N_ACTIONS=64):
    p_avg = mean_t(precondition);  e_avg = mean_t(effect)
    p_embed = p_avg @ Wp.T + bp;   e_embed = e_avg @ We.T + be
    p_t[b]  = W_trans[action[b]] @ p_embed[b]
    returns (p_t[:, None, :, None], e_embed)

Sharding strategy (hardcoded): sort samples by action on the host, split the
sorted batch into 8 contiguous chunks of 128 samples.  Each core receives its
chunk of precondition/effect, the K distinct expert matrices its chunk spans
(K = max over cores, zero-masked where unused), a one-hot sample->slot mask,
and full (replicated) projection weights.  The expert matvec is computed as
sum_k (p_embed * mask_k) @ W_sel[k].T accumulated in PSUM, which keeps the
program static SPMD while each core reads only ~K/64 of W_trans.

Precision: inputs/weights are cast to bf16 on the host (DMA and TensorE run
2x faster); all matmul accumulation is f32 in PSUM, pooling accumulates f32
on VectorE, and both outputs are written f32.

Device pipeline per core:
  1. x arrives as [J, S, T] bf16 so the mean-pool is a contiguous
     innermost-axis reduce_sum producing the pooled tile directly in the
     [j, s] layout the TensorEngine contraction needs (t is innermost and
     each partition reads 4KB-contiguous runs during the DMA)
  2. projections contract over j with the pooled tile as stationary operand
     and W.T streamed as the moving operand; the bias is folded into the same
     PSUM accumulation group as a rank-1 ones^T @ b matmul
  3. p_embed is PE-transposed to [d, s]; masked per expert slot; expert
     matmuls accumulate sum_k over K slots and 8 d-tiles into one PSUM pair
"""

import sys

try:  # concourse is on sys.path in the axon images; fall back to the repo
    import concourse.bass  # noqa: F401
except ImportError:  # pragma: no cover
    sys.path.insert(0, "/opt/trn_rl_repo")

import ml_dtypes
import numpy as np

import concourse.bass as bass
import concourse.mybir as mybir
import concourse.tile as tile
from concourse import bacc, bass2jax
from concourse.masks import make_identity

# Problem shape (hardcoded per contest rules)
B, T, J, D, NA = 1024, 16, 2048, 1024, 64
NC = 8            # cores
S = B // NC       # samples per core = 128
P = 128           # partitions
JT = J // P       # 16 j-tiles (input dim)
DT = D // P       # 8 d-tiles (embed dim)
NH = D // 512     # 2 psum-bank halves of the embed dim
G = 2             # j-tiles per pooling DMA chunk
F32 = mybir.dt.float32
BF16 = mybir.dt.bfloat16
NPBF16 = ml_dtypes.bfloat16

_kernel_cache: dict = {}


def _build(K: int):
    """Build the SPMD Bass program for K expert slots per core."""
    nc = bacc.Bacc(None, target_bir_lowering=False, debug=False, num_devices=NC)

    xp = nc.declare_dram_parameter("xp", [J, S, T], BF16, isOutput=False)
    xe = nc.declare_dram_parameter("xe", [J, S, T], BF16, isOutput=False)
    wp = nc.declare_dram_parameter("wp", [J, D], BF16, isOutput=False)
    we = nc.declare_dram_parameter("we", [J, D], BF16, isOutput=False)
    bp = nc.declare_dram_parameter("bp", [1, D], BF16, isOutput=False)
    be = nc.declare_dram_parameter("be", [1, D], BF16, isOutput=False)
    wsel = nc.declare_dram_parameter("wsel", [K, D, D], BF16, isOutput=False)
    mask = nc.declare_dram_parameter("mask", [K, P, S], BF16, isOutput=False)
    pt = nc.declare_dram_parameter("pt", [S, D], F32, isOutput=True)
    eo = nc.declare_dram_parameter("eo", [S, D], F32, isOutput=True)

    with tile.TileContext(nc) as tc:
        with (
            tc.tile_pool(name="xpool", bufs=4) as xpp,
            tc.tile_pool(name="pooledf", bufs=2) as pfp,
            tc.tile_pool(name="pooled", bufs=2) as poolp,
            tc.tile_pool(name="wproj", bufs=1) as wpp,
            tc.tile_pool(name="wselp", bufs=8) as wsp,
            tc.tile_pool(name="emb", bufs=2) as embp,
            tc.tile_pool(name="embT", bufs=1) as embTp,
            tc.tile_pool(name="small", bufs=4) as smallp,
            tc.tile_pool(name="maskp", bufs=1) as maskp,
            tc.tile_pool(name="xm", bufs=3) as xmp,
            tc.tile_pool(name="out", bufs=2) as outp,
            tc.tile_pool(name="ps", bufs=2, space="PSUM") as psp,
            tc.tile_pool(name="tps", bufs=2, space="PSUM") as tpsp,
            tc.tile_pool(name="ps2", bufs=1, space="PSUM") as ps2p,
        ):
            ident = smallp.tile([P, P], F32, tag="ident")
            make_identity(nc, ident[:])
            ones = smallp.tile([1, P], BF16, tag="ones")
            nc.gpsimd.memset(ones[:], 1.0)

            emb_tiles = {}
            for name, x_dram, w_dram, b_dram in (
                ("p", xp, wp, bp),
                ("e", xe, we, be),
            ):
                # ---- mean-pool over T: contiguous reduce, f32 accum ----
                x_t = x_dram.rearrange("(jg g p) s t -> jg p g s t", g=G, p=P)
                paTf = pfp.tile([P, JT * S], F32, tag="pooledf")
                for jg in range(JT // G):
                    xt = xpp.tile([P, G * S * T], BF16, tag="x")
                    nc.sync.dma_start(xt[:], x_t[jg])
                    nc.vector.reduce_sum(
                        paTf[:, jg * G * S : (jg + 1) * G * S],
                        xt[:].rearrange("p (g s t) -> p g s t", g=G, t=T),
                        axis=mybir.AxisListType.X,
                    )
                # cast pooled to bf16 for the matmul (mean 1/T folded in W)
                paT = poolp.tile([P, JT * S], BF16, tag="pooledT")
                nc.vector.tensor_copy(paT[:], paTf[:])

                bsb = smallp.tile([1, D], BF16, tag="bias")
                nc.sync.dma_start(bsb[:], b_dram[:])

                # ---- projection: emb[s, d] = paT.T @ (W.T/16) + ones.T @ b
                wbig = wpp.tile([P, JT * D], BF16, tag="w")
                nc.sync.dma_start(
                    wbig[:], w_dram.rearrange("(jt p) d -> p jt d", p=P)
                )
                emb = embp.tile([P, D], F32, tag="emb")
                psh = []
                for h in range(NH):
                    psh.append(psp.tile([P, 512], F32, tag="ps", name=f"ps{h}"))
                for jt in range(JT):
                    for h in range(NH):
                        nc.tensor.matmul(
                            psh[h][:],
                            paT[:, jt * S : (jt + 1) * S],
                            wbig[:, jt * D + h * 512 : jt * D + (h + 1) * 512],
                            start=(jt == 0),
                            stop=False,
                        )
                for h in range(NH):
                    # bias via rank-1 matmul: ones^T (1xS) @ b (1x512)
                    nc.tensor.matmul(
                        psh[h][:],
                        ones[:],
                        bsb[:, h * 512 : (h + 1) * 512],
                        start=False,
                        stop=True,
                    )
                    nc.scalar.activation(
                        emb[:, h * 512 : (h + 1) * 512],
                        psh[h][:],
                        mybir.ActivationFunctionType.Identity,
                    )
                emb_tiles[name] = emb

            # e_embed: straight out
            nc.sync.dma_start(eo[:], emb_tiles["e"][:])

            # ---- transpose p_embed [s, d] -> pembT [d, s] (bf16) ----
            pembT = embTp.tile([P, DT * S], BF16, tag="pembT")
            for dt in range(DT):
                tp = tpsp.tile([P, P], F32, tag="tps")
                nc.tensor.transpose(
                    tp[:], emb_tiles["p"][:, dt * P : (dt + 1) * P], ident[:]
                )
                nc.scalar.activation(
                    pembT[:, dt * S : (dt + 1) * S],
                    tp[:],
                    mybir.ActivationFunctionType.Identity,
                )

            # ---- expert transform: pt[s, i] = sum_k (pembT*m_k)^T @ Wsel_k
            msb = maskp.tile([P, K * S], BF16, tag="mask")
            nc.sync.dma_start(msb[:], mask.rearrange("k p s -> p k s"))

            DG = 2  # d-tiles per wsel DMA
            pspt = ps2p.tile([P, D], F32, tag="pspt")
            for k in range(K):
                xm = xmp.tile([P, DT * S], BF16, tag="xm")
                for dt in range(DT):
                    nc.vector.tensor_mul(
                        xm[:, dt * S : (dt + 1) * S],
                        pembT[:, dt * S : (dt + 1) * S],
                        msb[:, k * S : (k + 1) * S],
                    )
                wsel_k = wsel[k].rearrange("(dg g p) i -> dg p g i", g=DG, p=P)
                for dg in range(DT // DG):
                    wst = wsp.tile([P, DG * D], BF16, tag="wsel")
                    nc.scalar.dma_start(wst[:], wsel_k[dg])
                    for g in range(DG):
                        dt = dg * DG + g
                        first = k == 0 and dt == 0
                        last = k == K - 1 and dt == DT - 1
                        for h in range(NH):
                            nc.tensor.matmul(
                                pspt[:, h * 512 : (h + 1) * 512],
                                xm[:, dt * S : (dt + 1) * S],
                                wst[:, g * D + h * 512 : g * D + (h + 1) * 512],
                                start=first,
                                stop=last,
                            )
            ptsb = outp.tile([P, D], F32, tag="pt")
            nc.vector.tensor_copy(ptsb[:], pspt[:])
            nc.sync.dma_start(pt[:], ptsb[:])

    nc.compile()
    return nc


def _prep(precondition, effect, action, Wp, bp, We, be, W_trans):
    """Host-side routing + layout prep. Returns (in_maps, perm, K)."""
    act = np.asarray(action).astype(np.int64).ravel()
    perm = np.argsort(act, kind="stable")
    act_sorted = act[perm]

    # per-core distinct actions and sample->slot segmentation
    chunk_acts = act_sorted.reshape(NC, S)
    uniq = [np.unique(ca) for ca in chunk_acts]
    K = max(len(u) for u in uniq)

    xs_p = np.asarray(precondition, dtype=np.float32)[perm]
    xs_e = np.asarray(effect, dtype=np.float32)[perm]
    # [B, T, J] -> [J, B*T] (one cache-friendly 2D transpose) = [J, B, T],
    # then bf16; per-core slices below are contiguous row-chunk copies
    xt_p = np.ascontiguousarray(xs_p.reshape(B * T, J).T).astype(NPBF16)
    xt_p = xt_p.reshape(J, B, T)
    xt_e = np.ascontiguousarray(xs_e.reshape(B * T, J).T).astype(NPBF16)
    xt_e = xt_e.reshape(J, B, T)

    scale = np.float32(1.0 / T)
    wp_t = (np.ascontiguousarray(np.asarray(Wp, np.float32).T) * scale).astype(
        NPBF16
    )
    we_t = (np.ascontiguousarray(np.asarray(We, np.float32).T) * scale).astype(
        NPBF16
    )
    bp_ = np.asarray(bp, np.float32).reshape(1, D).astype(NPBF16)
    be_ = np.asarray(be, np.float32).reshape(1, D).astype(NPBF16)
    # rhs convention needs W^T ([j, i]); transpose once globally, then bf16
    Wt = np.ascontiguousarray(
        np.asarray(W_trans, np.float32).transpose(0, 2, 1)
    ).astype(NPBF16)

    in_maps = []
    for c in range(NC):
        ca = chunk_acts[c]
        u = uniq[c]
        sel = np.zeros((K, D, D), NPBF16)
        sel[: len(u)] = Wt[u]
        slot = np.searchsorted(u, ca)  # [S] slot index per sample
        m = np.zeros((K, S), NPBF16)
        m[slot, np.arange(S)] = 1.0
        # replicate mask across the 128 j-partitions: [K, P, S]
        mb = np.ascontiguousarray(np.broadcast_to(m[:, None, :], (K, P, S)))
        in_maps.append(
            {
                "xp": np.ascontiguousarray(xt_p[:, c * S : (c + 1) * S, :]),
                "xe": np.ascontiguousarray(xt_e[:, c * S : (c + 1) * S, :]),
                "wp": wp_t,
                "we": we_t,
                "bp": bp_,
                "be": be_,
                "wsel": sel,
                "mask": mb,
            }
        )
    return in_maps, perm, K


def kernel(precondition, effect, action, Wp, bp, We, be, W_trans):
    in_maps, perm, K = _prep(
        precondition, effect, action, Wp, bp, We, be, W_trans
    )
    nc = _kernel_cache.get(K)
    if nc is None:
        nc = _build(K)
        _kernel_cache[K] = nc

    results = bass2jax.run_bass_via_pjrt(nc, in_maps, n_cores=NC)

    p_sorted = np.concatenate([np.asarray(r["pt"]) for r in results], axis=0)
    e_sorted = np.concatenate([np.asarray(r["eo"]) for r in results], axis=0)
    inv = np.empty_like(perm)
    inv[perm] = np.arange(B)
    p_full = p_sorted[inv]
    e_full = e_sorted[inv]
    return (p_full[:, None, :, None].astype(np.float32),
            e_full.astype(np.float32))


# revision 8
# speedup vs baseline: 2.8845x; 1.0051x over previous
"""ActTransNet Trainium2 kernel: 8-core SPMD, routing done host-side.

Network (B=1024, T=16, INPUT_DIM=2048, DIM=1024, N_ACTIONS=64):
    p_avg = mean_t(precondition);  e_avg = mean_t(effect)
    p_embed = p_avg @ Wp.T + bp;   e_embed = e_avg @ We.T + be
    p_t[b]  = W_trans[action[b]] @ p_embed[b]
    returns (p_t[:, None, :, None], e_embed)

Sharding strategy (hardcoded): sort samples by action on the host, split the
sorted batch into 8 contiguous chunks of 128 samples.  Each core receives its
chunk of precondition/effect, the K distinct expert matrices its chunk spans
(K = max over cores, zero-masked where unused), a one-hot sample->slot mask,
and full (replicated) projection weights.  The expert matvec is computed as
sum_k (p_embed * mask_k) @ W_sel[k].T accumulated in PSUM, which keeps the
program static SPMD while each core reads only ~K/64 of W_trans.

Precision: inputs/weights are cast to bf16 on the host (DMA and TensorE run
2x faster); all matmul accumulation is f32 in PSUM, pooling accumulates f32
on VectorE, and both outputs are written f32.

Device pipeline per core (ordered so the p-side chain that feeds the
expert transform completes as early as possible; the e-side fills gaps):
  1. x_p arrives as [J, S, T] bf16, DMA'd in 2MB chunks alternating across
     both HWDGE rings; mean-pool is a contiguous innermost-axis reduce_sum
     producing per-chunk pooled tiles directly in [j, s] matmul layout
  2. proj-p contracts over j per chunk as pooled tiles arrive; bias is a
     rank-1 ones^T @ b matmul in the same PSUM accumulation group
  3. p_embed is PE-transposed to [d, s] bf16; per-slot masked copies feed
     the expert matmuls which accumulate K x 8 d-tiles into one PSUM pair,
     paced by the W_sel stream on the scalar HWDGE ring
  4. e-side (pool, proj, store) runs under the transform's DMA shadow
"""

import sys

try:  # concourse is on sys.path in the axon images; fall back to the repo
    import concourse.bass  # noqa: F401
except ImportError:  # pragma: no cover
    sys.path.insert(0, "/opt/trn_rl_repo")

import ml_dtypes
import numpy as np

import concourse.bass as bass
import concourse.mybir as mybir
import concourse.tile as tile
from concourse import bacc, bass2jax
from concourse.masks import make_identity

# Problem shape (hardcoded per contest rules)
B, T, J, D, NA = 1024, 16, 2048, 1024, 64
NC = 8            # cores
S = B // NC       # samples per core = 128
P = 128           # partitions
JT = J // P       # 16 j-tiles (input dim)
DT = D // P       # 8 d-tiles (embed dim)
NH = D // 512     # 2 psum-bank halves of the embed dim
G = 2             # j-tiles per pooling DMA chunk
NCH = JT // G     # pooling chunks per input
F32 = mybir.dt.float32
BF16 = mybir.dt.bfloat16
NPBF16 = ml_dtypes.bfloat16

_kernel_cache: dict = {}


def _build(K: int):
    """Build the SPMD Bass program for K expert slots per core."""
    nc = bacc.Bacc(None, target_bir_lowering=False, debug=False, num_devices=NC)

    xp = nc.declare_dram_parameter("xp", [J, S, T], BF16, isOutput=False)
    xe = nc.declare_dram_parameter("xe", [J, S, T], BF16, isOutput=False)
    wp = nc.declare_dram_parameter("wp", [J, D], BF16, isOutput=False)
    we = nc.declare_dram_parameter("we", [J, D], BF16, isOutput=False)
    bp = nc.declare_dram_parameter("bp", [1, D], BF16, isOutput=False)
    be = nc.declare_dram_parameter("be", [1, D], BF16, isOutput=False)
    wsel = nc.declare_dram_parameter("wsel", [K, D, D], BF16, isOutput=False)
    mask = nc.declare_dram_parameter("mask", [K, P, S], BF16, isOutput=False)
    pt = nc.declare_dram_parameter("pt", [S, D], F32, isOutput=True)
    eo = nc.declare_dram_parameter("eo", [S, D], F32, isOutput=True)

    with tile.TileContext(nc) as tc:
        with (
            tc.tile_pool(name="xpool", bufs=6) as xpp,
            tc.tile_pool(name="pooledf", bufs=3) as pfp,
            tc.tile_pool(name="pooled", bufs=2 * NCH) as poolp,
            tc.tile_pool(name="wproj", bufs=1) as wpp,
            tc.tile_pool(name="wselp", bufs=8) as wsp,
            tc.tile_pool(name="emb", bufs=2) as embp,
            tc.tile_pool(name="embT", bufs=1) as embTp,
            tc.tile_pool(name="small", bufs=4) as smallp,
            tc.tile_pool(name="maskp", bufs=1) as maskp,
            tc.tile_pool(name="xm", bufs=4) as xmp,
            tc.tile_pool(name="out", bufs=2) as outp,
            tc.tile_pool(name="ps", bufs=2, space="PSUM") as psp,
            tc.tile_pool(name="tps", bufs=2, space="PSUM") as tpsp,
            tc.tile_pool(name="ps2", bufs=1, space="PSUM") as ps2p,
        ):
            ident = smallp.tile([P, P], F32, tag="ident")
            make_identity(nc, ident[:])
            ones = smallp.tile([1, P], BF16, tag="ones")
            nc.gpsimd.memset(ones[:], 1.0)

            def pool_input(x_dram, engines):
                """DMA x chunks + reduce over T -> list of [P, G*S] bf16."""
                x_t = x_dram.rearrange("(jg g p) s t -> jg p g s t", g=G, p=P)
                tiles = []
                for jg in range(NCH):
                    xt = xpp.tile([P, G * S * T], BF16, tag="x", name=f"x{jg}")
                    engines[jg % len(engines)].dma_start(xt[:], x_t[jg])
                    pf = pfp.tile([P, G * S], F32, tag="pf", name=f"pf{jg}")
                    nc.vector.reduce_sum(
                        pf[:],
                        xt[:].rearrange("p (g s t) -> p g s t", g=G, t=T),
                        axis=mybir.AxisListType.X,
                    )
                    pa = poolp.tile([P, G * S], BF16, tag="pa", name=f"pa{jg}")
                    nc.vector.tensor_copy(pa[:], pf[:])  # cast to bf16
                    tiles.append(pa)
                return tiles

            def project(pa_tiles, w_dram, b_dram, out_dtype):
                """emb[s, d] = sum_jt pa.T @ (W.T/16) + ones.T @ b."""
                wbig = wpp.tile([P, JT * D], BF16, tag="w", name="wbig")
                nc.sync.dma_start(
                    wbig[:], w_dram.rearrange("(jt p) d -> p jt d", p=P)
                )
                bsb = smallp.tile([1, D], BF16, tag="bias", name="bsb")
                nc.sync.dma_start(bsb[:], b_dram[:])
                emb = embp.tile([P, D], out_dtype, tag="emb", name="emb")
                psh = []
                for h in range(NH):
                    psh.append(psp.tile([P, 512], F32, tag="ps", name=f"ps{h}"))
                for jt in range(JT):
                    pa = pa_tiles[jt // G]
                    gofs = (jt % G) * S
                    for h in range(NH):
                        nc.tensor.matmul(
                            psh[h][:],
                            pa[:, gofs : gofs + S],
                            wbig[:, jt * D + h * 512 : jt * D + (h + 1) * 512],
                            start=(jt == 0),
                            stop=False,
                        )
                for h in range(NH):
                    nc.tensor.matmul(
                        psh[h][:],
                        ones[:],
                        bsb[:, h * 512 : (h + 1) * 512],
                        start=False,
                        stop=True,
                    )
                    nc.scalar.activation(
                        emb[:, h * 512 : (h + 1) * 512],
                        psh[h][:],
                        mybir.ActivationFunctionType.Identity,
                    )
                return emb

            # ---- p-side chain first: it gates the expert transform ----
            pa_p = pool_input(xp, [nc.sync, nc.scalar])
            msb = maskp.tile([P, K * S], BF16, tag="mask")
            nc.scalar.dma_start(msb[:], mask.rearrange("k p s -> p k s"))
            emb_p = project(pa_p, wp, bp, F32)

            # transpose p_embed [s, d] -> pembT [d, s] (bf16)
            pembT = embTp.tile([P, DT * S], BF16, tag="pembT")
            for dt in range(DT):
                tp = tpsp.tile([P, P], F32, tag="tps", name=f"tp{dt}")
                nc.tensor.transpose(
                    tp[:], emb_p[:, dt * P : (dt + 1) * P], ident[:]
                )
                nc.scalar.activation(
                    pembT[:, dt * S : (dt + 1) * S],
                    tp[:],
                    mybir.ActivationFunctionType.Identity,
                )

            # ---- expert transform: pt[s, i] = sum_k (pembT*m_k)^T @ Wsel_k
            DG = 2  # d-tiles per wsel DMA
            pspt = ps2p.tile([P, D], F32, tag="pspt")
            for k in range(K):
                xm = xmp.tile([P, DT * S], BF16, tag="xm", name=f"xm{k}")
                for dt in range(DT):
                    nc.vector.tensor_mul(
                        xm[:, dt * S : (dt + 1) * S],
                        pembT[:, dt * S : (dt + 1) * S],
                        msb[:, k * S : (k + 1) * S],
                    )
                wsel_k = wsel[k].rearrange("(dg g p) i -> dg p g i", g=DG, p=P)
                for dg in range(DT // DG):
                    wst = wsp.tile([P, DG * D], BF16, tag="wsel", name=f"ws{k}_{dg}")
                    nc.scalar.dma_start(wst[:], wsel_k[dg])
                    for g in range(DG):
                        dt = dg * DG + g
                        first = k == 0 and dt == 0
                        last = k == K - 1 and dt == DT - 1
                        for h in range(NH):
                            nc.tensor.matmul(
                                pspt[:, h * 512 : (h + 1) * 512],
                                xm[:, dt * S : (dt + 1) * S],
                                wst[:, g * D + h * 512 : g * D + (h + 1) * 512],
                                start=first,
                                stop=last,
                            )

            # ---- e-side: runs under the transform's DMA shadow ----
            pa_e = pool_input(xe, [nc.sync])
            emb_e = project(pa_e, we, be, F32)
            nc.sync.dma_start(eo[:], emb_e[:])

            ptsb = outp.tile([P, D], F32, tag="pt")
            nc.vector.tensor_copy(ptsb[:], pspt[:])
            nc.sync.dma_start(pt[:], ptsb[:])

    nc.compile()
    return nc


def _prep(precondition, effect, action, Wp, bp, We, be, W_trans):
    """Host-side routing + layout prep. Returns (in_maps, perm, K)."""
    act = np.asarray(action).astype(np.int64).ravel()
    perm = np.argsort(act, kind="stable")
    act_sorted = act[perm]

    # per-core distinct actions and sample->slot segmentation
    chunk_acts = act_sorted.reshape(NC, S)
    uniq = [np.unique(ca) for ca in chunk_acts]
    K = max(len(u) for u in uniq)

    xs_p = np.asarray(precondition, dtype=np.float32)[perm]
    xs_e = np.asarray(effect, dtype=np.float32)[perm]
    # [B, T, J] -> [J, B*T] (one cache-friendly 2D transpose) = [J, B, T],
    # then bf16; per-core slices below are contiguous row-chunk copies
    xt_p = np.ascontiguousarray(xs_p.reshape(B * T, J).T).astype(NPBF16)
    xt_p = xt_p.reshape(J, B, T)
    xt_e = np.ascontiguousarray(xs_e.reshape(B * T, J).T).astype(NPBF16)
    xt_e = xt_e.reshape(J, B, T)

    scale = np.float32(1.0 / T)
    wp_t = (np.ascontiguousarray(np.asarray(Wp, np.float32).T) * scale).astype(
        NPBF16
    )
    we_t = (np.ascontiguousarray(np.asarray(We, np.float32).T) * scale).astype(
        NPBF16
    )
    bp_ = np.asarray(bp, np.float32).reshape(1, D).astype(NPBF16)
    be_ = np.asarray(be, np.float32).reshape(1, D).astype(NPBF16)
    # rhs convention needs W^T ([j, i]); transpose once globally, then bf16
    Wt = np.ascontiguousarray(
        np.asarray(W_trans, np.float32).transpose(0, 2, 1)
    ).astype(NPBF16)

    in_maps = []
    for c in range(NC):
        ca = chunk_acts[c]
        u = uniq[c]
        sel = np.zeros((K, D, D), NPBF16)
        sel[: len(u)] = Wt[u]
        slot = np.searchsorted(u, ca)  # [S] slot index per sample
        m = np.zeros((K, S), NPBF16)
        m[slot, np.arange(S)] = 1.0
        # replicate mask across the 128 j-partitions: [K, P, S]
        mb = np.ascontiguousarray(np.broadcast_to(m[:, None, :], (K, P, S)))
        in_maps.append(
            {
                "xp": np.ascontiguousarray(xt_p[:, c * S : (c + 1) * S, :]),
                "xe": np.ascontiguousarray(xt_e[:, c * S : (c + 1) * S, :]),
                "wp": wp_t,
                "we": we_t,
                "bp": bp_,
                "be": be_,
                "wsel": sel,
                "mask": mb,
            }
        )
    return in_maps, perm, K


def kernel(precondition, effect, action, Wp, bp, We, be, W_trans):
    in_maps, perm, K = _prep(
        precondition, effect, action, Wp, bp, We, be, W_trans
    )
    nc = _kernel_cache.get(K)
    if nc is None:
        nc = _build(K)
        _kernel_cache[K] = nc

    results = bass2jax.run_bass_via_pjrt(nc, in_maps, n_cores=NC)

    p_sorted = np.concatenate([np.asarray(r["pt"]) for r in results], axis=0)
    e_sorted = np.concatenate([np.asarray(r["eo"]) for r in results], axis=0)
    inv = np.empty_like(perm)
    inv[perm] = np.arange(B)
    p_full = p_sorted[inv]
    e_full = e_sorted[inv]
    return (p_full[:, None, :, None].astype(np.float32),
            e_full.astype(np.float32))
